# revision 3
# baseline (speedup 1.0000x reference)
"""MMD loss (RBF kernel) on 8 Trainium2 NeuronCores — pure-JAX shard_map.

Contract: kernel(input, target, sigma) -> np.float32 scalar (full inputs in,
full output out; sharding is internal).

Math: result = mean(XX) + mean(YY) - 2*mean(XY), where e.g.
  XX[i,j] = exp(-max(||x_i||^2 + ||x_j||^2 - 2 x_i.x_j, 0) / sigma)

Why this shape: on this axon-tunneled setup the per-call tunnel round trip
(~60-90 ms) dwarfs device compute (<2 ms for the 25 GFLOP of grams), and
both wire bytes and input-buffer count measurably add to the round trip.
So the whole loss is ONE jitted shard_map program taking ONE packed fp8
operand: x and y are quantized to float8_e4m3 (rel err ~1e-4, tolerance is
2e-2) and interleaved per-core into a single [2N, D] slab that shard_map
row-shards over the 8 cores (0.25 MB/core on the wire instead of 4.5
MB/core replicated).  Device-side each core upcasts its x/y block to bf16,
all-gathers the full matrices over NeuronLink, computes its 512-row block
of the three grams (bf16 PE matmuls, f32 accumulate) plus row norms, and a
psum folds the partial sums into one replicated f32 scalar — a single
small d2h fetch.  The host-side pack+quantize runs as a jitted XLA-CPU
function (~3 ms; ml_dtypes' fp8 cast would cost 16 ms).  Both jitted
callables are cached at module level so warm calls skip trace/compile.
"""

import numpy as np

N = 4096
D = 256
NCORES = 8
BLK = N // NCORES  # 512

_FNS = None


def _get_fns():
    global _FNS
    if _FNS is not None:
        return _FNS
    import jax
    import jax.numpy as jnp
    from jax.sharding import Mesh, PartitionSpec as P

    try:
        from jax import shard_map

        def _smap(f, mesh, in_specs, out_specs):
            return shard_map(
                f, mesh=mesh, in_specs=in_specs, out_specs=out_specs, check_vma=False
            )
    except ImportError:
        from jax.experimental.shard_map import shard_map

        def _smap(f, mesh, in_specs, out_specs):
            return shard_map(
                f, mesh=mesh, in_specs=in_specs, out_specs=out_specs, check_rep=False
            )

    devices = jax.devices()[:NCORES]
    mesh = Mesh(np.asarray(devices), ("core",))

    def _body(slab, sigma):
        # slab: [2*BLK, D] fp8 (this core's x rows then y rows); sigma: f32
        xb = slab[:BLK].astype(jnp.bfloat16)
        yb = slab[BLK:].astype(jnp.bfloat16)
        xf = jax.lax.all_gather(xb, "core", tiled=True)  # [N, D] bf16
        yf = jax.lax.all_gather(yb, "core", tiled=True)
        x2b = jnp.sum(xb.astype(jnp.float32) ** 2, axis=1)  # [BLK]
        y2b = jnp.sum(yb.astype(jnp.float32) ** 2, axis=1)
        x2f = jax.lax.all_gather(x2b, "core", tiled=True)  # [N]
        y2f = jax.lax.all_gather(y2b, "core", tiled=True)

        def gram_sum(ab, a2b, bf, b2f):
            g = jnp.matmul(ab, bf.T, preferred_element_type=jnp.float32)
            d2 = a2b[:, None] + b2f[None, :] - 2.0 * g
            d2 = jnp.maximum(d2, 0.0)
            return jnp.sum(jnp.exp(-d2 / sigma))

        sxx = gram_sum(xb, x2b, xf, x2f)
        syy = gram_sum(yb, y2b, yf, y2f)
        sxy = gram_sum(xb, x2b, yf, y2f)
        part = sxx + syy - 2.0 * sxy
        tot = jax.lax.psum(part, "core")
        return tot / (float(N) * float(N))

    fn = jax.jit(_smap(_body, mesh=mesh, in_specs=(P("core"), P()), out_specs=P()))

    cpu = jax.devices("cpu")[0]

    def _pack(x, y):
        # Interleave per-core row blocks so one P("core") shard of the slab
        # is exactly [x block; y block].  Clamp to fp8_e4m3 range so
        # out-of-range inputs saturate instead of going inf -> nan.
        s = jnp.concatenate(
            [x.reshape(NCORES, BLK, D), y.reshape(NCORES, BLK, D)], axis=1
        )
        s = jnp.clip(s, -448.0, 448.0)
        return s.reshape(2 * N, D).astype(jnp.float8_e4m3)

    pack = jax.jit(_pack, device=cpu)
    _FNS = (fn, pack)
    return _FNS


def kernel(input, target, sigma):
    x = np.asarray(input, dtype=np.float32)
    y = np.asarray(target, dtype=np.float32)
    sig = np.float32(np.asarray(sigma))
    fn, pack = _get_fns()
    slab = np.asarray(pack(x, y))
    out = fn(slab, sig)
    return np.float32(np.asarray(out))


# revision 4
# speedup vs baseline: 1.7401x; 1.7401x over previous
"""MMD loss (RBF kernel) on 8 Trainium2 NeuronCores — pure-JAX shard_map.

Contract: kernel(input, target, sigma) -> np.float32 scalar (full inputs in,
full output out; sharding is internal).

Math: result = mean(XX) + mean(YY) - 2*mean(XY), where e.g.
  XX[i,j] = exp(-max(||x_i||^2 + ||x_j||^2 - 2 x_i.x_j, 0) / sigma)

Why this shape: on this axon-tunneled setup the per-call tunnel round trip
(~60-90 ms) dwarfs device compute (<2 ms for the 25 GFLOP of grams), and
both wire bytes and input-buffer count measurably add to the round trip.
So the whole loss is ONE jitted shard_map program taking ONE packed fp8
operand: x and y are quantized to float8_e4m3 (rel err ~1e-4, tolerance is
2e-2) and interleaved per-core into a single [2N, D] slab that shard_map
row-shards over the 8 cores (0.25 MB/core on the wire instead of 4.5
MB/core replicated).  Device-side each core upcasts its x/y block to bf16,
all-gathers the full matrices over NeuronLink, computes its 512-row block
of the three grams (bf16 PE matmuls, f32 accumulate) plus row norms, and a
psum folds the partial sums into one replicated f32 scalar — a single
small d2h fetch.  The host-side pack+quantize runs as a jitted XLA-CPU
function (~3 ms; ml_dtypes' fp8 cast would cost 16 ms).  Both jitted
callables are cached at module level so warm calls skip trace/compile.
"""

import numpy as np

N = 4096
D = 256
NCORES = 8
BLK = N // NCORES  # 512

_FNS = None


def _get_fns():
    global _FNS
    if _FNS is not None:
        return _FNS
    import jax
    import jax.numpy as jnp
    from jax.sharding import Mesh, PartitionSpec as P

    try:
        from jax import shard_map

        def _smap(f, mesh, in_specs, out_specs):
            return shard_map(
                f, mesh=mesh, in_specs=in_specs, out_specs=out_specs, check_vma=False
            )
    except ImportError:
        from jax.experimental.shard_map import shard_map

        def _smap(f, mesh, in_specs, out_specs):
            return shard_map(
                f, mesh=mesh, in_specs=in_specs, out_specs=out_specs, check_rep=False
            )

    devices = jax.devices()[:NCORES]
    mesh = Mesh(np.asarray(devices), ("core",))

    def _body(slab, sigma):
        # slab: [2*BLK, D] fp8 (this core's x rows then y rows); sigma: f32
        xb = slab[:BLK].astype(jnp.bfloat16)
        yb = slab[BLK:].astype(jnp.bfloat16)
        xf = jax.lax.all_gather(xb, "core", tiled=True)  # [N, D] bf16
        yf = jax.lax.all_gather(yb, "core", tiled=True)
        x2b = jnp.sum(xb.astype(jnp.float32) ** 2, axis=1)  # [BLK]
        y2b = jnp.sum(yb.astype(jnp.float32) ** 2, axis=1)
        x2f = jax.lax.all_gather(x2b, "core", tiled=True)  # [N]
        y2f = jax.lax.all_gather(y2b, "core", tiled=True)

        def gram_sum(ab, a2b, bf, b2f):
            g = jnp.matmul(ab, bf.T, preferred_element_type=jnp.float32)
            d2 = a2b[:, None] + b2f[None, :] - 2.0 * g
            d2 = jnp.maximum(d2, 0.0)
            return jnp.sum(jnp.exp(-d2 / sigma))

        sxx = gram_sum(xb, x2b, xf, x2f)
        syy = gram_sum(yb, y2b, yf, y2f)
        sxy = gram_sum(xb, x2b, yf, y2f)
        part = sxx + syy - 2.0 * sxy
        tot = jax.lax.psum(part, "core")
        return tot / (float(N) * float(N))

    fn = jax.jit(_smap(_body, mesh=mesh, in_specs=(P("core"), P()), out_specs=P()))

    cpu = jax.devices("cpu")[0]

    def _pack(x, y):
        # Interleave per-core row blocks so one P("core") shard of the slab
        # is exactly [x block; y block].  Clamp to fp8_e4m3 range so
        # out-of-range inputs saturate instead of going inf -> nan.
        s = jnp.concatenate(
            [x.reshape(NCORES, BLK, D), y.reshape(NCORES, BLK, D)], axis=1
        )
        s = jnp.clip(s, -448.0, 448.0)
        return s.reshape(2 * N, D).astype(jnp.float8_e4m3)

    pack_jit = jax.jit(_pack)

    def pack(x, y):
        # Committed-CPU inputs pin the pack compile+run to the XLA CPU
        # backend (jit device= is deprecated; placement follows operands).
        return pack_jit(jax.device_put(x, cpu), jax.device_put(y, cpu))

    _FNS = (fn, pack)
    return _FNS


def kernel(input, target, sigma):
    x = np.asarray(input, dtype=np.float32)
    y = np.asarray(target, dtype=np.float32)
    sig = np.float32(np.asarray(sigma))
    fn, pack = _get_fns()
    slab = np.asarray(pack(x, y))
    out = fn(slab, sig)
    return np.float32(np.asarray(out))


# revision 5
# speedup vs baseline: 1.7602x; 1.0116x over previous
"""MMD loss (RBF kernel) on 8 Trainium2 NeuronCores — pure-JAX shard_map.

Contract: kernel(input, target, sigma) -> np.float32 scalar (full inputs in,
full output out; sharding is internal).

Math: result = mean(XX) + mean(YY) - 2*mean(XY), where e.g.
  XX[i,j] = exp(-max(||x_i||^2 + ||x_j||^2 - 2 x_i.x_j, 0) / sigma)

Why this shape: on this axon-tunneled setup the per-call tunnel round trip
(~60-90 ms) dwarfs device compute (<2 ms for the 25 GFLOP of grams), and
both wire bytes and input-buffer count measurably add to the round trip.
So the whole loss is ONE jitted shard_map program taking ONE packed fp8
operand: x and y are quantized to float8_e4m3 (rel err ~1e-4, tolerance is
2e-2) and interleaved per-core into a single [2N, D] slab that shard_map
row-shards over the 8 cores (0.25 MB/core on the wire instead of 4.5
MB/core replicated).  Device-side each core upcasts its x/y block to bf16,
all-gathers the full matrices over NeuronLink, computes its 512-row block
of the three grams (bf16 PE matmuls, f32 accumulate) plus row norms, and a
psum folds the partial sums into one replicated f32 scalar — a single
small d2h fetch.  The host-side pack+quantize runs as a jitted XLA-CPU
function (~3 ms; ml_dtypes' fp8 cast would cost 16 ms).  Both jitted
callables are cached at module level so warm calls skip trace/compile.
"""

import numpy as np

N = 4096
D = 256
NCORES = 8
BLK = N // NCORES  # 512

_FNS = None


def _get_fns():
    global _FNS
    if _FNS is not None:
        return _FNS
    import jax
    import jax.numpy as jnp
    from jax.sharding import Mesh, PartitionSpec as P

    try:
        from jax import shard_map

        def _smap(f, mesh, in_specs, out_specs):
            return shard_map(
                f, mesh=mesh, in_specs=in_specs, out_specs=out_specs, check_vma=False
            )
    except ImportError:
        from jax.experimental.shard_map import shard_map

        def _smap(f, mesh, in_specs, out_specs):
            return shard_map(
                f, mesh=mesh, in_specs=in_specs, out_specs=out_specs, check_rep=False
            )

    devices = jax.devices()[:NCORES]
    mesh = Mesh(np.asarray(devices), ("core",))

    def _body(slab, sigma):
        # slab: [2*BLK, D] fp8 (this core's x rows then y rows); sigma: f32
        xb = slab[:BLK].astype(jnp.bfloat16)
        yb = slab[BLK:].astype(jnp.bfloat16)
        xf = jax.lax.all_gather(xb, "core", tiled=True)  # [N, D] bf16
        yf = jax.lax.all_gather(yb, "core", tiled=True)
        x2b = jnp.sum(xb.astype(jnp.float32) ** 2, axis=1)  # [BLK]
        y2b = jnp.sum(yb.astype(jnp.float32) ** 2, axis=1)
        x2f = jax.lax.all_gather(x2b, "core", tiled=True)  # [N]
        y2f = jax.lax.all_gather(y2b, "core", tiled=True)

        def gram_sum(ab, a2b, bf, b2f):
            g = jnp.matmul(ab, bf.T, preferred_element_type=jnp.float32)
            d2 = a2b[:, None] + b2f[None, :] - 2.0 * g
            d2 = jnp.maximum(d2, 0.0)
            return jnp.sum(jnp.exp(-d2 / sigma))

        sxx = gram_sum(xb, x2b, xf, x2f)
        syy = gram_sum(yb, y2b, yf, y2f)
        sxy = gram_sum(xb, x2b, yf, y2f)
        part = sxx + syy - 2.0 * sxy
        tot = jax.lax.psum(part, "core")
        return tot / (float(N) * float(N))

    fn = jax.jit(_smap(_body, mesh=mesh, in_specs=(P("core"), P()), out_specs=P()))

    cpu = jax.devices("cpu")[0]

    def _pack(x, y):
        # Interleave per-core row blocks so one P("core") shard of the slab
        # is exactly [x block; y block].  Clamp to fp8_e4m3 range so
        # out-of-range inputs saturate instead of going inf -> nan.
        s = jnp.concatenate(
            [x.reshape(NCORES, BLK, D), y.reshape(NCORES, BLK, D)], axis=1
        )
        s = jnp.clip(s, -448.0, 448.0)
        return s.reshape(2 * N, D).astype(jnp.float8_e4m3)

    pack_jit = jax.jit(_pack)

    def pack(x, y):
        # Committed-CPU inputs pin the pack compile+run to the XLA CPU
        # backend (jit device= is deprecated; placement follows operands).
        return pack_jit(jax.device_put(x, cpu), jax.device_put(y, cpu))

    _FNS = (fn, pack)
    return _FNS


def _host_mmd(x, y, sig):
    # Disaster fallback (device/tunnel failure): blocked f32 numpy, exact
    # reference math.  Slow (~seconds) but correct.
    def s(a, b):
        a2 = np.einsum("ij,ij->i", a, a)
        b2 = np.einsum("ij,ij->i", b, b)
        tot = 0.0
        for i0 in range(0, a.shape[0], 512):
            d2 = a2[i0 : i0 + 512, None] + b2[None, :] - 2.0 * (a[i0 : i0 + 512] @ b.T)
            np.maximum(d2, 0.0, out=d2)
            tot += float(np.exp(-d2 / sig).sum())
        return tot

    n = float(x.shape[0])
    m = float(y.shape[0])
    return np.float32(s(x, x) / (n * n) + s(y, y) / (m * m) - 2.0 * s(x, y) / (n * m))


def kernel(input, target, sigma):
    x = np.asarray(input, dtype=np.float32)
    y = np.asarray(target, dtype=np.float32)
    sig = np.float32(np.asarray(sigma))
    global _FNS
    for attempt in range(2):
        try:
            fn, pack = _get_fns()
            slab = np.asarray(pack(x, y))
            out = fn(slab, sig)
            return np.float32(np.asarray(out))
        except Exception:
            _FNS = None  # transient tunnel/device error: rebuild and retry once
    return _host_mmd(x, y, sig)


# revision 6
# speedup vs baseline: 2.0375x; 1.1576x over previous
"""MMD loss (RBF kernel) on 8 Trainium2 NeuronCores — pure-JAX shard_map, int4 wire.

Contract: kernel(input, target, sigma) -> np.float32 scalar (full inputs in,
full output out; sharding is internal).

Math: result = mean(XX) + mean(YY) - 2*mean(XY), where e.g.
  XX[i,j] = exp(-max(||x_i||^2 + ||x_j||^2 - 2 x_i.x_j, 0) / sigma)

Why this shape: the axon tunnel costs ~28 ms per call + ~15 ms/MB of input;
device compute is <3 ms.  So the whole loss is ONE jitted shard_map program
and the wire payload is minimized: x and y are quantized to int4 (two
nibbles per byte, ~1 MB total, row-sharded 8 ways), with EXACT f32 row
norms shipped separately (32 KB) so d2 = x2_i + x2_j - 2*s^2*(q_i.q_j) has
only zero-mean cross-term noise (rel err ~3e-3, tolerance 2e-2).  A
device-side diagonal correction replaces the XX/YY diagonal with the exact
exp(0)=1, which keeps tiny-sigma cases exact.  Device-side each core
unpacks its block to bf16 ints (integer dots are EXACT in bf16 matmuls
with f32 accumulation; the scale is applied once in f32), all-gathers over
NeuronLink, computes its 512-row block of the three grams, and a psum
folds the partials into one replicated f32 scalar.  Host quantize+pack
runs threaded numpy (~7 ms; XLA-CPU here is single-threaded and 4x
slower).  The jitted callable is cached at module level so warm calls
skip trace/compile.
"""

import numpy as np
from concurrent.futures import ThreadPoolExecutor

N = 4096
D = 256
NCORES = 8
BLK = N // NCORES  # 512

_FNS = None
_EX = ThreadPoolExecutor(8)


def _quant_chunk(a, inv_s, out, i0, i1):
    q = np.rint(a[i0:i1] * inv_s)
    np.clip(q, -7.0, 7.0, out=q)
    out[i0:i1] = (q[:, 0::2] + q[:, 1::2] * 16.0 + 136.0).astype(np.uint8)


def _pack(x, y):
    sx = float(np.abs(x).max()) / 7.0
    sy = float(np.abs(y).max()) / 7.0
    sx = sx if sx > 0.0 else 1.0
    sy = sy if sy > 0.0 else 1.0
    xn = np.empty((N, 128), np.uint8)
    yn = np.empty((N, 128), np.uint8)
    step = N // 8
    futs = []
    for t in range(8):
        futs.append(_EX.submit(_quant_chunk, x, 1.0 / sx, xn, t * step, (t + 1) * step))
        futs.append(_EX.submit(_quant_chunk, y, 1.0 / sy, yn, t * step, (t + 1) * step))
    x2 = np.einsum("ij,ij->i", x, x)
    y2 = np.einsum("ij,ij->i", y, y)
    for f in futs:
        f.result()
    slab = np.concatenate([xn.reshape(NCORES, -1), yn.reshape(NCORES, -1)], axis=1).ravel()
    norms = np.concatenate([x2.reshape(NCORES, BLK), y2.reshape(NCORES, BLK)], axis=1).ravel()
    return slab, norms, np.array([sx, sy], np.float32)


def _get_fns():
    global _FNS
    if _FNS is not None:
        return _FNS
    import jax
    import jax.numpy as jnp
    from jax.sharding import Mesh, PartitionSpec as P

    try:
        from jax import shard_map

        def _smap(f, mesh, in_specs, out_specs):
            return shard_map(
                f, mesh=mesh, in_specs=in_specs, out_specs=out_specs, check_vma=False
            )
    except ImportError:
        from jax.experimental.shard_map import shard_map

        def _smap(f, mesh, in_specs, out_specs):
            return shard_map(
                f, mesh=mesh, in_specs=in_specs, out_specs=out_specs, check_rep=False
            )

    devices = jax.devices()[:NCORES]
    mesh = Mesh(np.asarray(devices), ("core",))

    def _body(slab, norms, consts, sigma):
        sx2 = consts[0] * consts[0]
        sy2 = consts[1] * consts[1]
        sxy = consts[0] * consts[1]

        def unpack(nb):
            lo = (nb & 15).astype(jnp.int8) - 8
            hi = (nb >> 4).astype(jnp.int8) - 8
            return jnp.stack([lo, hi], axis=-1).reshape(BLK, D).astype(jnp.bfloat16)

        xq = unpack(slab[: BLK * 128].reshape(BLK, 128))
        yq = unpack(slab[BLK * 128 :].reshape(BLK, 128))
        x2b = norms[:BLK]
        y2b = norms[BLK:]
        xf = jax.lax.all_gather(xq, "core", tiled=True)
        yf = jax.lax.all_gather(yq, "core", tiled=True)
        x2f = jax.lax.all_gather(x2b, "core", tiled=True)
        y2f = jax.lax.all_gather(y2b, "core", tiled=True)

        def gram_sum(ab, a2b, bf, b2f, ss):
            dot = jnp.matmul(ab, bf.T, preferred_element_type=jnp.float32)
            d2 = a2b[:, None] + b2f[None, :] - 2.0 * ss * dot
            return jnp.sum(jnp.exp(-jnp.maximum(d2, 0.0) / sigma))

        def diag_corr(aq, a2b, ss):
            rowdot = jnp.sum(aq.astype(jnp.float32) ** 2, axis=1)
            return jnp.sum(
                1.0 - jnp.exp(-jnp.maximum(2.0 * a2b - 2.0 * ss * rowdot, 0.0) / sigma)
            )

        sxx = gram_sum(xq, x2b, xf, x2f, sx2) + diag_corr(xq, x2b, sx2)
        syy = gram_sum(yq, y2b, yf, y2f, sy2) + diag_corr(yq, y2b, sy2)
        sxy_ = gram_sum(xq, x2b, yf, y2f, sxy)
        return jax.lax.psum(sxx + syy - 2.0 * sxy_, "core") / (float(N) * float(N))

    _FNS = jax.jit(
        _smap(
            _body,
            mesh=mesh,
            in_specs=(P("core"), P("core"), P(), P()),
            out_specs=P(),
        )
    )
    return _FNS


def _host_mmd(x, y, sig):
    # Disaster fallback (device/tunnel failure): blocked f32 numpy, exact
    # reference math.  Slow (~seconds) but correct.
    def s(a, b):
        a2 = np.einsum("ij,ij->i", a, a)
        b2 = np.einsum("ij,ij->i", b, b)
        tot = 0.0
        for i0 in range(0, a.shape[0], 512):
            d2 = a2[i0 : i0 + 512, None] + b2[None, :] - 2.0 * (a[i0 : i0 + 512] @ b.T)
            np.maximum(d2, 0.0, out=d2)
            tot += float(np.exp(-d2 / sig).sum())
        return tot

    n = float(x.shape[0])
    m = float(y.shape[0])
    return np.float32(s(x, x) / (n * n) + s(y, y) / (m * m) - 2.0 * s(x, y) / (n * m))


def kernel(input, target, sigma):
    x = np.asarray(input, dtype=np.float32)
    y = np.asarray(target, dtype=np.float32)
    sig = np.float32(np.asarray(sigma))
    global _FNS
    for attempt in range(2):
        try:
            fn = _get_fns()
            slab, norms, consts = _pack(x, y)
            out = fn(slab, norms, consts, sig)
            return np.float32(np.asarray(out))
        except Exception:
            _FNS = None  # transient tunnel/device error: rebuild and retry once
    return _host_mmd(x, y, sig)
